# revision 22
# baseline (speedup 1.0000x reference)
"""Trainium2 Bass kernel for a teacher-forced/autoregressive GRU decoder.

Problem: B=256, T=1024, D=64, H=512 GRU with teacher forcing for t < cutoff
and mean-feedback autoregression for t >= cutoff, decoder producing
(mean, std) per step.

Strategy: pure data-parallel over 8 NeuronCores (32 sequences/core).
Everything on-chip lives in a transposed layout (feature on partitions,
batch on the free dim) so the per-step gate math runs on 128-partition
tiles.  Matmuls keep the weights stationary (bf16, FWL) and stream the
32-wide batch.  gi/gh/biases accumulate in PSUM so the sigmoid reads a
single finished tile.  The time loop is a For_i with a U-step unrolled
body; the decoder output of step t feeds x of step t+1 through a
persistent SBUF tile, which makes phase 1 and phase 2 bodies identical
except for where the gi matmul's moving operand comes from.
"""

import math

import numpy as np
import ml_dtypes

import concourse.bass as bass
import concourse.mybir as mybir
from concourse.tile import TileContext

F32 = mybir.dt.float32
BF16 = mybir.dt.bfloat16
AF = mybir.ActivationFunctionType
OP = mybir.AluOpType

# ---------------------------------------------------------------------------
# This walrus (neuronx-cc) build rejects instructions carrying too many
# sync-wait commands.  Tile attaches global-clock waits to loop-reset and
# context-exit drains, overflowing the limit for any nontrivial kernel.
# Post-pass: split the wait list of any over-limit instruction across a
# chain of same-engine NOPs inserted immediately before it.
# ---------------------------------------------------------------------------
_MAX_WAITS = 1  # probed empirically: this walrus accepts one sync-wait/inst


def _split_overlimit_waits(nc, max_waits=_MAX_WAITS):
    n_split = 0
    for f in nc.m.functions:
        for bb in f.blocks:
            insts = bb.instructions
            i = 0
            while i < len(insts):
                inst = insts[i]
                si = inst.sync_info
                if si is not None and si.on_wait and len(si.on_wait) > max_waits:
                    waits = list(si.on_wait)
                    chunks = [
                        waits[j : j + max_waits]
                        for j in range(0, len(waits), max_waits)
                    ]
                    inst.sync_info = mybir.SyncInfo(
                        on_wait=chunks[-1], on_update=list(si.on_update or [])
                    )
                    for k, ch in enumerate(chunks[:-1]):
                        nop = mybir.InstNoOp(
                            name=nc.get_next_instruction_name(), ins=[], outs=[]
                        )
                        nop.engine = inst.engine
                        nop.sync_info = mybir.SyncInfo(on_wait=ch, on_update=[])
                        insts.insert(i + k, nop)
                    i += len(chunks) - 1
                    n_split += 1
                i += 1
    return n_split

B, T, D, H = 256, 1024, 64, 512
NCORES = 8
BL = B // NCORES          # 32 sequences per core
KIN = D + 2               # x(64) + ts(1) + ones(1) rows of the input operand
NM = (3 * H) // 128       # 12 gate chunks of 128
NK = H // 128             # 4 hidden chunks of 128
NRZ = (2 * H) // 128      # 8 chunks belonging to the r|z gates
STD_LB = 1e-3


def build_gru_bass(t_len: int, cutoff: int, unroll: int, repeats: int = 1):
    """Emit the Bass module for one core (BL sequences, t_len steps).

    repeats > 1 wraps the whole computation in an outer loop (state re-init
    included) so on-device time can be measured from wall clock; outputs are
    identical to repeats=1."""
    nc = bass.Bass()

    X = nc.declare_dram_parameter("X", [KIN, t_len * BL], BF16, isOutput=False)
    WIH = nc.declare_dram_parameter("WIH", [KIN, NM * 128], BF16, isOutput=False)
    WHH = nc.declare_dram_parameter("WHH", [128, NM * NK * 128], BF16, isOutput=False)
    WDEC = nc.declare_dram_parameter("WDEC", [128, NK * 128], BF16, isOutput=False)
    BHHN = nc.declare_dram_parameter("BHHN", [1, NK * 128], BF16, isOutput=False)
    BDEC = nc.declare_dram_parameter("BDEC", [128, 1], F32, isOutput=False)
    OUT = nc.declare_dram_parameter("OUT", [128, t_len * BL], F32, isOutput=True)

    with TileContext(nc) as tc:
        with (
            tc.tile_pool(name="const", bufs=1) as cpool,
            tc.tile_pool(name="state", bufs=1) as spool,
            tc.tile_pool(name="xblk", bufs=2) as xpool,
            tc.tile_pool(name="oblk", bufs=2) as opool,
            tc.tile_pool(name="gates", bufs=2) as gpool,
            tc.tile_pool(name="psum", bufs=2, space="PSUM") as ppool,
        ):
            # ---- persistent constants ----
            wih_t = cpool.tile([KIN, NM * 128], BF16)
            whh_t = cpool.tile([128, NM * NK * 128], BF16)
            wdec_t = cpool.tile([128, NK * 128], BF16)
            bhhn_t = cpool.tile([1, NK * 128], BF16)
            bdec_t = cpool.tile([128, 1], F32)
            ones_t = cpool.tile([1, BL], BF16)

            nc.sync.dma_start(wih_t[:], WIH[:])
            nc.sync.dma_start(whh_t[:], WHH[:])
            nc.sync.dma_start(wdec_t[:], WDEC[:])
            nc.sync.dma_start(bhhn_t[:], BHHN[:])
            nc.sync.dma_start(bdec_t[:], BDEC[:])
            nc.vector.memset(ones_t[:], 1.0)

            # ---- persistent state (ping-pong across steps, bf16 only) ----
            hbf = [
                spool.tile([128, NK * BL], BF16, name=f"hbf_{i}", tag=f"hbf_{i}")
                for i in range(2)
            ]
            x_tile = spool.tile([KIN, BL], BF16)

            def emit_state_init():
                for i in range(2):
                    nc.vector.memset(hbf[i][:], 0.0)
                nc.vector.memset(x_tile[:], 0.0)

            def emit_gi(rhs_x):
                """gi matmuls into a fresh GI psum tile (self-contained groups:
                one start+stop mm per bank region, sequential in PE order).
                The rz half is bounced to SBUF (DVE ops may read at most one
                PSUM operand, and rzsum already reads RZ from PSUM)."""
                GI = ppool.tile([128, NM * BL], F32, tag="gi", name="GI")
                for m in range(NM):
                    nc.tensor.matmul(
                        GI[:, m * BL : (m + 1) * BL],
                        wih_t[:, m * 128 : (m + 1) * 128],
                        rhs_x,
                        start=True,
                        stop=True,
                    )
                gi_sb = gpool.tile([128, NRZ * BL], F32, tag="gi_sb", name="gi_sb")
                nc.scalar.activation(gi_sb[:], GI[:, 0 : NRZ * BL], AF.Copy)
                return GI, gi_sb

            def emit_step(s, xblk, oblk, phase2, gi_pipe):
                """One GRU step. s = step index inside the unrolled body.

                gi_pipe: GI psum tile of this step (phase 1: computed by the
                previous step's emission so it filled the gate window)."""
                cur, nxt = s % 2, (s + 1) % 2
                ssl = slice(s * BL, (s + 1) * BL)

                if phase2:
                    # feedback operand: x rows were written by the previous
                    # step's decoder; ts+ones rows come from the streamed block
                    nc.gpsimd.tensor_copy(x_tile[D : D + 2, :], xblk[D : D + 2, ssl])
                    GI, gi_sb = emit_gi(x_tile[:])
                else:
                    GI, gi_sb = gi_pipe

                RZ = ppool.tile([128, NRZ * BL], F32, tag="rz")
                HN = ppool.tile([128, NK * BL], F32, tag="hn")
                DEC = ppool.tile([128, BL], F32, tag="dec")

                # gh (rz chunks): pure W_hh @ h accumulation, bank-sequential
                for m in range(NRZ):
                    for k in range(NK):
                        nc.tensor.matmul(
                            RZ[:, m * BL : (m + 1) * BL],
                            whh_t[:, (m * NK + k) * 128 : (m * NK + k + 1) * 128],
                            hbf[cur][:, k * BL : (k + 1) * BL],
                            start=(k == 0),
                            stop=(k == NK - 1),
                        )
                # gh (n chunks) + b_hh_n seed via a 1-row matmul
                for c in range(NK):
                    m = NRZ + c
                    out_ap = HN[:, c * BL : (c + 1) * BL]
                    nc.tensor.matmul(
                        out_ap,
                        bhhn_t[:, c * 128 : (c + 1) * 128],
                        ones_t[:],
                        start=True,
                        stop=False,
                    )
                    for k in range(NK):
                        nc.tensor.matmul(
                            out_ap,
                            whh_t[:, (m * NK + k) * 128 : (m * NK + k + 1) * 128],
                            hbf[cur][:, k * BL : (k + 1) * BL],
                            start=False,
                            stop=(k == NK - 1),
                        )

                # ---- gates (rzsum/sigmoid/w/q overlap the n-chunk matmuls) ----
                rzsum = gpool.tile([128, NRZ * BL], F32, tag="rzsum")
                nc.vector.scalar_tensor_tensor(
                    rzsum[:], RZ[:], 0.0, gi_sb[:], OP.bypass, OP.add
                )
                rz_s = gpool.tile([128, NRZ * BL], F32, tag="rz_s")
                nc.scalar.activation(rz_s[:], rzsum[:], AF.Sigmoid)
                r_ap = rz_s[:, 0 : NK * BL]
                z_ap = rz_s[:, NK * BL : NRZ * BL]

                w_s = gpool.tile([128, NK * BL], F32, tag="w_s")  # 1 - z
                nc.vector.tensor_scalar(w_s[:], z_ap, -1.0, 1.0, OP.mult, OP.add)
                q_s = gpool.tile([128, NK * BL], F32, tag="q_s")  # z * h
                nc.gpsimd.tensor_tensor(q_s[:], z_ap, hbf[cur][:], OP.mult)

                t1 = gpool.tile([128, NK * BL], F32, tag="t1")
                nc.vector.scalar_tensor_tensor(
                    t1[:], HN[:], 0.0, r_ap, OP.bypass, OP.mult
                )
                t2 = gpool.tile([128, NK * BL], F32, tag="t2")
                nc.vector.scalar_tensor_tensor(
                    t2[:], t1[:], 0.0, GI[:, NRZ * BL : NM * BL], OP.bypass, OP.add
                )
                n_s = gpool.tile([128, NK * BL], F32, tag="n_s")
                nc.scalar.activation(n_s[:], t2[:], AF.Tanh)

                u_s = gpool.tile([128, NK * BL], F32, tag="u_s")
                nc.vector.tensor_tensor(u_s[:], n_s[:], w_s[:], OP.mult)
                # h_{t+1} = n*(1-z) + z*h, written directly as bf16
                nc.vector.tensor_tensor(hbf[nxt][:], u_s[:], q_s[:], OP.add)

                # phase 1: next step's gi is teacher-forced -> emit it here so
                # the PE chews on it while the gate chain runs (the last step of
                # the body skips it; the next iteration's prologue provides it)
                gi_next = None
                if not phase2 and s < unroll - 1:
                    gi_next = emit_gi(xblk[:, (s + 1) * BL : (s + 2) * BL])

                # ---- decoder on h_{t+1} ----
                for k in range(NK):
                    nc.tensor.matmul(
                        DEC[:],
                        wdec_t[:, k * 128 : (k + 1) * 128],
                        hbf[nxt][:, k * BL : (k + 1) * BL],
                        start=(k == 0),
                        stop=(k == NK - 1),
                    )
                # mean feedback for the next step's x (harmless in phase 1)
                nc.scalar.activation(
                    x_tile[0:D, :], DEC[0:D, :], AF.Identity, bias=bdec_t[0:D, 0:1]
                )
                # outputs (off the critical chain, on DVE)
                nc.vector.tensor_scalar(
                    oblk[0:D, ssl], DEC[0:D, :], bdec_t[0:D, 0:1], None, OP.add
                )
                nc.vector.tensor_scalar(
                    oblk[D:128, ssl],
                    DEC[D:128, :],
                    bdec_t[D:128, 0:1],
                    STD_LB,
                    OP.add,
                    OP.max,
                )
                return gi_next

            def emit_phase(t0, t1, phase2):
                n_iter = (t1 - t0) // unroll
                if n_iter == 0:
                    return
                blk = unroll * BL
                with tc.For_i(t0 * BL, t1 * BL, blk) as iv:
                    xblk = xpool.tile([KIN, blk], BF16, tag="xblk")
                    nc.sync.dma_start(xblk[:], X[:, bass.ds(iv, blk)])
                    oblk = opool.tile([128, blk], F32, tag="oblk")
                    gi_pipe = None if phase2 else emit_gi(xblk[:, 0:BL])
                    for s in range(unroll):
                        gi_pipe = emit_step(s, xblk, oblk, phase2, gi_pipe)
                    nc.sync.dma_start(OUT[:, bass.ds(iv, blk)], oblk[:])

            def emit_all():
                emit_state_init()
                emit_phase(0, cutoff, phase2=False)
                emit_phase(cutoff, t_len, phase2=True)

            if repeats > 1:
                with tc.For_i(0, repeats, 1):
                    emit_all()
            else:
                emit_all()

    return nc


def pack_core_inputs(xs_c, ts_c, t_len):
    """xs_c (BL, T, D), ts_c (BL, T, 1) -> X (KIN, T*BL) bf16."""
    xin = np.empty((KIN, t_len, BL), np.float32)
    xin[0:D] = xs_c.transpose(2, 1, 0)
    xin[D] = ts_c[:, :, 0].T
    xin[D + 1] = 1.0
    return xin.reshape(KIN, t_len * BL).astype(ml_dtypes.bfloat16)


def pack_weights(W_ih, W_hh, b_ih, b_hh, W_dec, b_dec):
    wih_l = np.empty((KIN, 3 * H), np.float32)
    wih_l[0:D] = W_ih[:, 1 : 1 + D].T
    wih_l[D] = W_ih[:, 0]
    bias = np.concatenate([b_ih[: 2 * H] + b_hh[: 2 * H], b_ih[2 * H :]])
    wih_l[D + 1] = bias

    # WHH[p, (m*NK+k)*128 + c] = W_hh[m*128 + c, k*128 + p]
    whh_l = W_hh.reshape(NM, 128, NK, 128).transpose(3, 0, 2, 1).reshape(128, -1)
    # WDEC[p, k*128 + m'] = W_dec[m', k*128 + p]
    wdec_l = W_dec.reshape(128, NK, 128).transpose(2, 1, 0).reshape(128, -1)

    bf = ml_dtypes.bfloat16
    return {
        "WIH": wih_l.astype(bf),
        "WHH": np.ascontiguousarray(whh_l).astype(bf),
        "WDEC": np.ascontiguousarray(wdec_l).astype(bf),
        "BHHN": b_hh[2 * H :].reshape(1, -1).astype(bf),
        "BDEC": np.asarray(b_dec, np.float32).reshape(128, 1),
    }


def unpack_output(out_c, t_len):
    """OUT (128, T*BL) f32 -> (BL, T, 2D)."""
    return np.asarray(out_c, np.float32).reshape(128, t_len, BL).transpose(2, 1, 0)


def _pick_unroll(cutoff, t_len):
    for u in (16, 8, 4, 2, 1):
        if cutoff % u == 0 and (t_len - cutoff) % u == 0:
            return u
    return 1


def kernel(
    xs, ts, W_ih, W_hh, b_ih, b_hh, W_dec, b_dec, cutoff, trace=False, repeats=1
):
    from concourse.bass_utils import run_bass_kernel_spmd

    xs = np.asarray(xs, np.float32)
    ts = np.asarray(ts, np.float32)
    cutoff = int(cutoff)
    t_len = xs.shape[1]
    assert xs.shape == (B, t_len, D) and 0 < cutoff <= t_len

    nc = build_gru_bass(t_len, cutoff, _pick_unroll(cutoff, t_len), repeats=repeats)
    # required for the walrus build in this container; CoreSim paths skip it
    _split_overlimit_waits(nc)

    wmap = pack_weights(
        np.asarray(W_ih, np.float32),
        np.asarray(W_hh, np.float32),
        np.asarray(b_ih, np.float32),
        np.asarray(b_hh, np.float32),
        np.asarray(W_dec, np.float32),
        np.asarray(b_dec, np.float32),
    )
    in_maps = []
    for c in range(NCORES):
        sl = slice(c * BL, (c + 1) * BL)
        in_maps.append({"X": pack_core_inputs(xs[sl], ts[sl], t_len), **wmap})

    res = run_bass_kernel_spmd(nc, in_maps, core_ids=list(range(NCORES)), trace=trace)
    out = np.concatenate(
        [unpack_output(res.results[c]["OUT"], t_len) for c in range(NCORES)], axis=0
    )
    if trace:
        kernel.last_exec_time_ns = res.exec_time_ns
        kernel.last_results = res
    return out
